# revision 44
# baseline (speedup 1.0000x reference)
"""Trainium2 Bass kernel: dense transformer attention block (QKV proj + RoPE +
GQA causal attention + output proj), tensor-parallel over 8 NeuronCores.

Sharding: heads split across cores (4 Q heads + 1 KV head per core). Each core
computes its QKV shard for all tokens, runs attention for its heads, then a
PARTIAL output projection contracted over its own 512 attention rows for ALL
4096 output columns; the host sums the 8 fp32 partials. No on-device
collective at all.

v5: collective elimination. Hardware measurement shows a NEFF that contains
ANY collective runs its matmul stream at ~1.93 GHz for the whole execution,
while the identical stream without collectives sustains ~2.37 GHz (a latched
~22% clock penalty -- even one AllGather that completes in the first 100us
leaves the rest of the kernel throttled). Swapping the o-proj AllGather
(32MB/core gathered) for host-summed row-partials keeps FLOPs and weight
bytes identical, moves 64MB of fp32 partial writes per core (2KB lines,
~50GB/s, harmless), and restores the fast clock. The o-proj is interleaved
per attention pair, so phase B shrinks to the last pair + drain.

Also retained from v4: Q/K never round-trip through HBM (RoPE writes the
SBUF attention tiles directly), and all DMA stays off the scalar queue while
EXPs are in flight.

v6 (~753us): the 4MB o-proj weight load is deferred behind the qkv weights
(it was first on the scalar queue, delaying the first matmul to ~30us); the
o-proj PSUM rotates through both PSUM pools (4-deep) so the matmul chain
never WAR-waits on the evacuation copy two jts back; the causal-diagonal key
tile computes at half width; the softmax reciprocal uses the fast
single-instruction variant (pair-boundary critical path); pt/res pipelines
deepened with the SBUF freed by dropping the reciprocal scratch.

v7 (~738us): every pair's o-proj now runs as PE filler inside the NEXT
pair's ACT-bound attention (DVE-only evacuation in filler mode so copies
never queue ahead of EXPs on the ACT engine), with leftovers draining after
the pair; both score pairs of a key tile are emitted before PV(kt-1) to
double the EXP->PV slack; all weight loads are split so each chain's slices
land progressively.
"""

from contextlib import ExitStack

import numpy as np
import ml_dtypes

import concourse.bass as bass
from concourse import bacc
import concourse.tile as tile
import concourse.mybir as mybir
from concourse.bass_utils import run_bass_kernel_spmd

F32 = mybir.dt.float32
F32R = mybir.dt.float32r
BF16 = mybir.dt.bfloat16
EXP = mybir.ActivationFunctionType.Exp

N_CORES = 8
N_HEADS = 32
N_KV_HEADS = 8
D = 128          # head dim
HID = 4096
B = 2
S = 2048
T = B * S        # 4096 tokens
ROPE_BASE = 10000.0

HL = N_HEADS // N_CORES          # 4 local Q heads per core

TC = 512                         # token chunk for the QKV projection phase
QC = 256                         # query chunk in attention
N_HT = HID // 128                # 32 hidden tiles
N_QC = S // QC                   # 8 query chunks per batch
N_JT = HID // 128                # 32 output-column tiles
N_CK = T // QC                   # 16 query chunks overall


def _emit(tc_ctx, xt, wqkvt, wot, ropes, out_t):
    nc = tc_ctx.nc
    n_ch = T // TC               # 8 qkv chunks
    n_kt = S // 128              # 16 k-tiles per batch

    with ExitStack() as es:
        const_pool = es.enter_context(tc_ctx.tile_pool(name="const", bufs=1))
        # All-ones stationary: one matmul both sums over the key partition
        # axis and broadcasts the sums across all 128 partitions.
        ones_mat = const_pool.tile([128, 128], F32R)
        # Diagonal causal masks for a packed 4-slot pt tile:
        # mask4[d_off][k, slot, q] = 1.0 iff q - k - 128*d_off >= 0.
        mask4 = [const_pool.tile([128, 4, QC], BF16, name=f"mask4_{d_off}")
                 for d_off in range(2)]
        for d_off in range(2):
            nc.vector.memset(mask4[d_off], 1.0)
        # memset on a float32r tile fails the ISA check; copy from the
        # all-ones bf16 tile instead.
        nc.vector.tensor_copy(ones_mat, mask4[0][:, 0, 0:128])
        for d_off in range(2):
            for slot in range(4):
                nc.gpsimd.affine_select(
                    out=mask4[d_off][:, slot, :],
                    in_=mask4[d_off][:, slot, :],
                    compare_op=mybir.AluOpType.is_ge,
                    fill=0.0,
                    base=-128 * d_off,
                    pattern=[[1, QC]],
                    channel_multiplier=-1,
                )
        # Warm the ACT exp table before attention needs it.
        act_warm = const_pool.tile([128, 1], F32)
        nc.scalar.activation(act_warm, ones_mat[:, 0:1], EXP)

        # Q/K/V live entirely in SBUF (written by phase A, read by attention).
        qpool = es.enter_context(tc_ctx.tile_pool(name="p2_q", bufs=4))
        kvpool = es.enter_context(tc_ctx.tile_pool(name="p2_kv", bufs=2))
        ps_s = es.enter_context(
            tc_ctx.tile_pool(name="p2_ps_s", bufs=2, space="PSUM"))
        ps_o = es.enter_context(
            tc_ctx.tile_pool(name="p2_ps_o", bufs=1, space="PSUM"))
        ps_op = es.enter_context(
            tc_ctx.tile_pool(name="p3_ps", bufs=2, space="PSUM"))
        ptpool = es.enter_context(tc_ctx.tile_pool(name="p2_pt", bufs=6))
        cspool = es.enter_context(tc_ctx.tile_pool(name="p2_cs", bufs=2))
        mpool = es.enter_context(tc_ctx.tile_pool(name="p2_misc", bufs=1))
        atpool = es.enter_context(tc_ctx.tile_pool(name="p2_attn", bufs=2))
        respool = es.enter_context(tc_ctx.tile_pool(name="p3_res", bufs=6))
        wopool = es.enter_context(tc_ctx.tile_pool(name="p3_wo", bufs=1))

        # o-proj weights for this core's 512 attention rows, all 4096 cols.
        # (Loaded inside phase A, AFTER the qkv weights: this 4MB transfer
        # ahead of them was delaying the very first matmul by ~10us.)
        wo_sb = wopool.tile([128, HL, N_JT, 128], BF16)

        kvq = {}
        qtiles = {}
        attn_tiles = {}

        def alloc_q(b, qc):
            q_t = qpool.tile([128, HL, QC], BF16, tag="q",
                             name=f"q_t{b}_{qc}")
            qtiles[(b, qc)] = q_t
            return q_t

        def alloc_kv(b):
            k_sb = kvpool.tile([128, S], BF16, tag="k", name=f"k_sb{b}")
            v_sb = kvpool.tile([128, n_kt, 128], BF16, tag="v",
                               name=f"v_sb{b}")
            kvq[b] = (k_sb, v_sb)

        def emit_attn(b, qc, at_pair, sub, filler=None, prologue=None):
            k_sb, v_sb = kvq[b]
            q_sb = qtiles.pop((b, qc))
            kt_max = 2 * qc + 2
            pso = [ps_o.tile([128, 2, QC], F32, tag=f"pso{hp}",
                             name=f"pso{hp}_{b}_{qc}")
                   for hp in range(2)]
            colsum = cspool.tile([128, 4, QC], F32R)
            pts = {}

            def emit_scores(kt, hp):
                d_off = kt - 2 * qc
                if d_off == 1:
                    # Last key tile: only the upper half of the query chunk
                    # is causally visible -- compute at half width.
                    ps = ps_s.tile([128, 2, 128], F32, tag="ps",
                                   name=f"psh_{qc}_{kt}_{hp}")
                    nc.tensor.matmul(
                        ps,
                        lhsT=k_sb[:, kt * 128:(kt + 1) * 128],
                        rhs=q_sb[:, hp * 2:hp * 2 + 2, 128:QC],
                        start=True,
                        stop=True,
                    )
                    pt = ptpool.tile([128, 2, 128], BF16, tag="pt",
                                     name=f"pth_{qc}_{kt}_{hp}")
                    nc.scalar.activation(pt, ps, EXP)
                    # mask: q' >= k, i.e. the d_off=0 mask's first 128 cols.
                    nc.vector.tensor_mul(
                        pt, pt, mask4[0][:, 2 * hp:2 * hp + 2, 0:128]
                    )
                    return pt
                # One N=512 matmul covers both heads of the pair.
                ps = ps_s.tile([128, 2, QC], F32, tag="ps")
                nc.tensor.matmul(
                    ps,
                    lhsT=k_sb[:, kt * 128:(kt + 1) * 128],
                    rhs=q_sb[:, hp * 2:hp * 2 + 2, :],
                    start=True,
                    stop=True,
                )
                pt = ptpool.tile([128, 2, QC], BF16, tag="pt")
                nc.scalar.activation(pt, ps, EXP)
                if d_off == 0:
                    nc.vector.tensor_mul(
                        pt, pt, mask4[0][:, 2 * hp:2 * hp + 2, :]
                    )
                return pt

            def emit_pv(kt):
                half = (kt == kt_max - 1 and kt_max >= 2)
                for hp in range(2):
                    pt = pts.pop(kt)[hp] if hp == 1 else pts[kt][hp]
                    if half:
                        # Half-width PV into the upper query columns; the
                        # full-width accumulation group was closed at
                        # kt_max-2, so skip the group check.
                        nc.tensor.matmul(
                            pso[hp][:, :, 128:QC],
                            lhsT=v_sb[:, kt, :],
                            rhs=pt,
                            start=False,
                            stop=True,
                            skip_group_check=True,
                        )
                        nc.vector.tensor_add(
                            colsum[:, hp * 2:hp * 2 + 2, 128:QC],
                            colsum[:, hp * 2:hp * 2 + 2, 128:QC],
                            pt,
                        )
                        continue
                    nc.tensor.matmul(
                        pso[hp],
                        lhsT=v_sb[:, kt, :],
                        rhs=pt,
                        start=(kt == 0),
                        stop=(kt == max(kt_max - 2, 0)),
                    )
                    if kt == 0:
                        nc.vector.tensor_copy(
                            colsum[:, hp * 2:hp * 2 + 2, :], pt
                        )
                    else:
                        nc.vector.tensor_add(
                            colsum[:, hp * 2:hp * 2 + 2, :],
                            colsum[:, hp * 2:hp * 2 + 2, :],
                            pt,
                        )

            for kt in range(kt_max):
                # Emit both score pairs of kt before PV(kt-1) so the
                # in-order PE stream gives every EXP two matmuls of slack.
                cur = [emit_scores(kt, 0), emit_scores(kt, 1)]
                if kt >= 1:
                    emit_pv(kt - 1)
                pts[kt] = cur
                if prologue is not None and kt == 1:
                    # The previous query chunk's finalize (ones-matmul /
                    # reciprocal / normalize) runs here, hidden behind this
                    # chunk's independent score stream.
                    prologue()
                    prologue = None
                if filler is not None and kt % 2 == 1:
                    # PE filler (o-proj jt blocks of the previous pair)
                    # while the serial EXP chain catches up.
                    filler()
            emit_pv(kt_max - 1)

            # Partition-reduce + broadcast the denominators (into the score
            # PSUM pool -- scores are drained by now), then normalize.
            def finalize():
                sums_bc = [ps_s.tile([128, 2, QC], F32, tag="ps",
                                     name=f"sums{hp}_{b}_{qc}")
                           for hp in range(2)]
                for hp in range(2):
                    nc.tensor.matmul(
                        sums_bc[hp],
                        lhsT=ones_mat,
                        rhs=colsum[:, hp * 2:hp * 2 + 2, :],
                        start=True,
                        stop=True,
                    )
                recip = mpool.tile([128, 4, QC], F32, tag="recip")
                for hp in range(2):
                    # Fast single-instruction reciprocal (~12-bit): plenty
                    # for a softmax denominator, halves the DVE latency.
                    nc.vector.reciprocal_approx_fast(
                        recip[:, hp * 2:hp * 2 + 2, :], sums_bc[hp])
                for hp in range(2):
                    nc.vector.tensor_mul(
                        at_pair[:, hp * 2:hp * 2 + 2, sub, :],
                        pso[hp], recip[:, hp * 2:hp * 2 + 2, :]
                    )

            return finalize

        def emit_oproj_jt(pair, at_pair, jt, act_ok=True):
            if True:
                # Alternate PSUM pools: a 4-deep rotation so the matmul
                # chain never WAR-waits on the evacuation copy two jts back.
                # The first four jts stay on ps_op: the ps_s rotation still
                # holds sums_bc, whose reciprocal read is slow.
                if jt % 2 == 0 or jt < 4:
                    po = ps_op.tile([128, 2, QC], F32, tag="po",
                                    name=f"po_{pair}_{jt}")
                else:
                    po = ps_s.tile([128, 2, QC], F32, tag="ps",
                                   name=f"po_{pair}_{jt}")
                for hh in range(HL):
                    nc.tensor.matmul(
                        po,
                        lhsT=wo_sb[:, hh, jt, :],
                        rhs=at_pair[:, hh],
                        start=(hh == 0),
                        stop=(hh == HL - 1),
                    )
                res = respool.tile([128, 2, QC], F32, tag="res",
                                   name=f"res_{pair}_{jt}")
                # Alternate the PSUM evacuation between DVE and ACT (the ACT
                # engine is idle during o-proj segments -- but when this jt is
                # filler inside an attention pair, ACT is pacing the EXPs and
                # must not be given copies).
                if jt % 2 == 0 or not act_ok:
                    nc.vector.tensor_copy(res, po)
                else:
                    nc.scalar.activation(
                        res, po, mybir.ActivationFunctionType.Copy)
                # 2KB lines: [128 j, 2 chunks, 256 t] fp32 per write,
                # alternating queues so the 32-write burst doesn't delay the
                # next chunk's x/rope loads behind it on the sync ring.
                eng = nc.sync if jt % 2 == 0 else nc.scalar
                eng.dma_start(
                    out=out_t.ap()[jt, :, 2 * pair:2 * pair + 2, :],
                    in_=res,
                )

        def emit_oproj_pair(pair):
            # Partial o-proj for this pair's 512 tokens, contracted over the
            # core's 512 attention rows (4 head-tiles x 2 query chunks).
            at_pair = attn_tiles.pop(pair)
            for jt in range(N_JT):
                emit_oproj_jt(pair, at_pair, jt)

        def emit_attn_pair(pair, filler=None):
            b, p = divmod(pair, 4)
            at_pair = atpool.tile([128, HL, 2, QC], BF16, tag="attn",
                                  name=f"attn_{pair}")
            attn_tiles[pair] = at_pair
            fin0 = emit_attn(b, 2 * p, at_pair, 0, filler)
            fin1 = emit_attn(b, 2 * p + 1, at_pair, 1, filler,
                             prologue=fin0)
            return fin1

        # ------ Phase A: QKV projection + RoPE, attention + o-proj
        # interleaved one pair behind -------
        with tc_ctx.tile_pool(name="p1_w", bufs=1) as wpool, \
             tc_ctx.tile_pool(name="p1_x", bufs=3) as xpool, \
             tc_ctx.tile_pool(name="p1_rope", bufs=2) as rpool, \
             tc_ctx.tile_pool(name="p1_ps", bufs=2, space="PSUM") as pspool, \
             tc_ctx.tile_pool(name="p1_sh", bufs=1) as shpool:
            wq_sb = wpool.tile([128, HL + 2, N_HT, 128], BF16)
            for ot in range(HL + 2):
                # Weights on the scalar-engine queue so the first X chunk
                # (sync queue) lands in parallel. Split ot=0 finely so the
                # very first matmuls start as soon as a slice arrives.
                if ot <= 1:
                    for hq in range(4):
                        nc.scalar.dma_start(
                            out=wq_sb[:, ot, hq * 8:(hq + 1) * 8],
                            in_=wqkvt.ap()[:, ot, hq * 8:(hq + 1) * 8],
                        )
                else:
                    for hq in range(2):
                        nc.scalar.dma_start(
                            out=wq_sb[:, ot, hq * 16:(hq + 1) * 16],
                            in_=wqkvt.ap()[:, ot, hq * 16:(hq + 1) * 16],
                        )
            # o-proj weights after all qkv weights; first needed ~chunk 2.
            nc.scalar.dma_start(out=wo_sb, in_=wot.ap())
            for ch in range(n_ch):
                b, p = divmod(ch, S // TC)
                if p == 0:
                    alloc_kv(b)
                q_lo = alloc_q(b, 2 * p)
                q_hi = alloc_q(b, 2 * p + 1)
                # x in two 16-ht halves (16KB/partition each, triple
                # buffered) to fit SBUF alongside the o-proj weights.
                x_half = []
                for half in range(2):
                    xh = xpool.tile([128, N_HT // 2, TC], BF16, tag="x",
                                    name=f"x_{ch}_{half}")
                    lo = half * (N_HT // 2)
                    if ch == 0 and half == 0:
                        for piece in range(2):
                            nc.sync.dma_start(
                                out=xh[:, piece * 8:(piece + 1) * 8, :],
                                in_=xt.ap()[:, ch,
                                            lo + piece * 8:lo + (piece + 1) * 8,
                                            :],
                            )
                    else:
                        nc.sync.dma_start(out=xh,
                                          in_=xt.ap()[:, ch, lo:lo + 16, :])
                    x_half.append(xh)

                def x_sl(h):
                    return x_half[h // 16][:, h % 16, :]

                rope_sb = rpool.tile([128, 4, TC], BF16)
                nc.sync.dma_start(out=rope_sb, in_=ropes.ap()[:, ch])
                for ot in range(HL + 2):
                    if ot == HL + 1:
                        # V head, computed TRANSPOSED ([token, d]) by making
                        # the x slice the stationary operand: no HBM round
                        # trip and no DMA-transpose instructions.
                        psv = pspool.tile([128, 4, 128], F32, tag="ps")
                        # h-major: each x half is fully consumed at the
                        # loop midpoint, so the next chunk's x loads (which
                        # WAR on these tiles) can start earlier.
                        for h in range(N_HT):
                            for sub in range(4):
                                nc.tensor.matmul(
                                    psv[:, sub, :],
                                    lhsT=x_sl(h)[:, sub * 128:(sub + 1) * 128],
                                    rhs=wq_sb[:, ot, h, :],
                                    start=(h == 0 and sub == 0),
                                    stop=(h == N_HT - 1 and sub == 3),
                                )
                        v_sb = kvq[b][1]
                        nc.vector.tensor_copy(
                            v_sb[:, 4 * p:4 * p + 4, :], psv
                        )
                        continue
                    ps = pspool.tile([128, TC], F32, tag="ps")
                    for h in range(N_HT):
                        nc.tensor.matmul(
                            ps,
                            lhsT=wq_sb[:, ot, h, :],
                            rhs=x_sl(h),
                            start=(h == 0),
                            stop=(h == N_HT - 1),
                        )
                    # RoPE for Q (ot<HL, scaled tables) and K (ot==HL),
                    # written straight into the SBUF attention tiles.
                    ci = 0 if ot < HL else 2
                    # sh = rotate_half(ps) * sin  (sign folded into sin)
                    sh = shpool.tile([128, TC], F32, tag="sh")
                    nc.vector.tensor_mul(
                        sh[0:64, :], ps[64:128, :], rope_sb[0:64, ci + 1, :]
                    )
                    nc.vector.tensor_mul(
                        sh[64:128, :], ps[0:64, :],
                        rope_sb[64:128, ci + 1, :]
                    )
                    tmp = shpool.tile([128, TC], F32, tag="tmp")
                    nc.vector.tensor_mul(tmp, ps, rope_sb[:, ci, :])
                    if ot < HL:
                        nc.vector.tensor_add(
                            q_lo[:, ot, :], tmp[:, 0:QC], sh[:, 0:QC]
                        )
                        nc.vector.tensor_add(
                            q_hi[:, ot, :], tmp[:, QC:TC], sh[:, QC:TC]
                        )
                    else:
                        k_sb_b = kvq[b][0]
                        nc.vector.tensor_add(
                            k_sb_b[:, p * TC:(p + 1) * TC], tmp, sh
                        )
                # Chunk ch complete: run attention for the pair that
                # became ready one chunk ago, feeding the 2-back pair's
                # o-proj jt-blocks in as PE filler inside the (ACT-bound)
                # EXP stretches; leftovers drain right after.
                if ch >= 2:
                    at_fill = attn_tiles.pop(ch - 2)
                    fqs = list(range(N_JT))

                    def fill_fn(pair=ch - 2, at=at_fill, fqs=fqs):
                        if fqs:
                            emit_oproj_jt(pair, at, fqs.pop(0), act_ok=False)

                    fin = emit_attn_pair(ch - 1, filler=fill_fn)
                    for _ in range(2):
                        if fqs:
                            emit_oproj_jt(ch - 2, at_fill, fqs.pop(0))
                    fin()
                    while fqs:
                        emit_oproj_jt(ch - 2, at_fill, fqs.pop(0))
                elif ch >= 1:
                    emit_attn_pair(ch - 1)()

        # -------- Phase B: the last pair, with the pending pair's o-proj
        # jt-blocks as PE filler inside its ACT-bound attention ----------
        at_prev = attn_tiles.pop(n_ch - 2)
        fq = list(range(N_JT))

        def filler():
            if fq:
                emit_oproj_jt(n_ch - 2, at_prev, fq.pop(0), act_ok=False)

        fin_last = emit_attn_pair(n_ch - 1, filler=filler)
        for _ in range(2):
            if fq:
                emit_oproj_jt(n_ch - 2, at_prev, fq.pop(0))
        fin_last()
        while fq:
            emit_oproj_jt(n_ch - 2, at_prev, fq.pop(0))
        emit_oproj_pair(n_ch - 1)


def _build_program():
    nc = bacc.Bacc("TRN2", target_bir_lowering=False, debug=False,
                   num_devices=N_CORES)
    xt = nc.declare_dram_parameter("xt", [128, T // TC, N_HT, TC], BF16,
                                   isOutput=False)
    wqkvt = nc.declare_dram_parameter("wqkvt", [128, HL + 2, N_HT, 128], BF16,
                                      isOutput=False)
    wot = nc.declare_dram_parameter("wot", [128, HL, N_JT, 128], BF16,
                                    isOutput=False)
    ropes = nc.declare_dram_parameter("ropes", [128, T // TC, 4, TC], BF16,
                                      isOutput=False)
    # fp32 partial o-proj: [jt, j-in-tile, query-chunk, t]; host sums cores.
    out_t = nc.declare_dram_parameter("out_t", [N_JT, 128, N_CK, QC], F32,
                                      isOutput=True)

    with tile.TileContext(nc) as tc_ctx:
        _emit(tc_ctx, xt, wqkvt, wot, ropes, out_t)
    nc.finalize()
    return nc


def _host_inputs(hidden_states, w_qkv, w_o):
    """Shard + transpose inputs for the 8 cores; returns in_maps."""
    X = np.asarray(hidden_states, dtype=np.float32).reshape(T, HID)
    # [p, ch, ht, tc] tiled layout so every DMA line is contiguous.
    xt = np.ascontiguousarray(
        X.reshape(T // TC, TC, N_HT, 128).transpose(3, 0, 2, 1)
    ).astype(ml_dtypes.bfloat16)

    # RoPE tables in [d, t] layout with rotate-half sign folded into sin and
    # the attention scale folded into the Q tables.
    inv_freq = 1.0 / (ROPE_BASE ** (np.arange(0, D, 2, dtype=np.float32) / D))
    pos = np.arange(S, dtype=np.float32)
    freqs = np.outer(pos, inv_freq)                      # (S, D/2)
    emb = np.concatenate([freqs, freqs], axis=-1)        # (S, D)
    cos = np.cos(emb).T.astype(np.float32)               # (D, S)
    sin = np.sin(emb).T.astype(np.float32)
    sgn = np.concatenate([-np.ones(D // 2), np.ones(D // 2)]).astype(np.float32)
    sins = sgn[:, None] * sin
    cos_t = np.tile(cos, (1, B))                         # (D, T)
    sins_t = np.tile(sins, (1, B))
    scale = np.float32(D ** -0.5)
    ropes = np.stack([cos_t * scale, sins_t * scale, cos_t, sins_t], axis=0)
    ropes = np.ascontiguousarray(
        ropes.reshape(4, 128, T // TC, TC).transpose(1, 2, 0, 3)
    ).astype(ml_dtypes.bfloat16)

    w_qkv = np.asarray(w_qkv, dtype=np.float32)
    w_o = np.asarray(w_o, dtype=np.float32)
    q_sz = N_HEADS * D
    kv_sz = N_KV_HEADS * D
    in_maps = []
    for c in range(N_CORES):
        qr = w_qkv[c * HL * D:(c + 1) * HL * D]
        kr = w_qkv[q_sz + c * D:q_sz + (c + 1) * D]
        vr = w_qkv[q_sz + kv_sz + c * D:q_sz + kv_sz + (c + 1) * D]
        w_shard = np.concatenate([qr, kr, vr], axis=0)           # (768, HID)
        wqkvt_c = np.ascontiguousarray(
            w_shard.reshape(HL + 2, 128, N_HT, 128).transpose(3, 0, 2, 1)
        ).astype(ml_dtypes.bfloat16)
        # o-proj slice: this core's 512 attention rows, all 4096 columns,
        # laid out [d-part, hh, jt, j].
        wo_rows = w_o[:, c * HL * D:(c + 1) * HL * D]            # (4096, 512)
        wot_c = np.ascontiguousarray(
            wo_rows.T.reshape(HL, 128, N_JT, 128).transpose(1, 0, 2, 3)
        ).astype(ml_dtypes.bfloat16)
        in_maps.append({
            "xt": xt, "wqkvt": wqkvt_c, "wot": wot_c, "ropes": ropes,
        })
    return in_maps


def _run(hidden_states, w_qkv, w_o, trace=False, tmpdir=None):
    in_maps = _host_inputs(hidden_states, w_qkv, w_o)
    nc = _build_program()
    res = run_bass_kernel_spmd(nc, in_maps, list(range(N_CORES)),
                               trace=trace, tmpdir=tmpdir)
    acc = np.zeros((N_JT, 128, N_CK, QC), dtype=np.float32)
    for c in range(N_CORES):
        acc += np.asarray(res.results[c]["out_t"])
    # [jt, j, ck, t] -> [j(4096), t(4096)] -> [B, S, HID]
    out_jt = acc.reshape(N_JT * 128, N_CK * QC)
    out = np.ascontiguousarray(out_jt.T).reshape(B, S, HID).astype(np.float32)
    return out, res


def kernel(hidden_states, w_qkv, w_o):
    out, _ = _run(hidden_states, w_qkv, w_o, trace=False)
    return out


# revision 45
# speedup vs baseline: 1.1934x; 1.1934x over previous
"""Trainium2 Bass kernel: dense transformer attention block (QKV proj + RoPE +
GQA causal attention + output proj), tensor-parallel over 8 NeuronCores.

Sharding: heads split across cores (4 Q heads + 1 KV head per core). Each core
computes its QKV shard for all tokens, runs attention for its heads, then a
PARTIAL output projection contracted over its own 512 attention rows for ALL
4096 output columns; the host sums the 8 fp32 partials. No on-device
collective at all.

v5: collective elimination. Hardware measurement shows a NEFF that contains
ANY collective runs its matmul stream at ~1.93 GHz for the whole execution,
while the identical stream without collectives sustains ~2.37 GHz (a latched
~22% clock penalty -- even one AllGather that completes in the first 100us
leaves the rest of the kernel throttled). Swapping the o-proj AllGather
(32MB/core gathered) for host-summed row-partials keeps FLOPs and weight
bytes identical, moves 64MB of fp32 partial writes per core (2KB lines,
~50GB/s, harmless), and restores the fast clock. The o-proj is interleaved
per attention pair, so phase B shrinks to the last pair + drain.

Also retained from v4: Q/K never round-trip through HBM (RoPE writes the
SBUF attention tiles directly), and all DMA stays off the scalar queue while
EXPs are in flight.

v6 (~753us): the 4MB o-proj weight load is deferred behind the qkv weights
(it was first on the scalar queue, delaying the first matmul to ~30us); the
o-proj PSUM rotates through both PSUM pools (4-deep) so the matmul chain
never WAR-waits on the evacuation copy two jts back; the causal-diagonal key
tile computes at half width; the softmax reciprocal uses the fast
single-instruction variant (pair-boundary critical path); pt/res pipelines
deepened with the SBUF freed by dropping the reciprocal scratch.

v7 (~738us): every pair's o-proj now runs as PE filler inside the NEXT
pair's ACT-bound attention (DVE-only evacuation in filler mode so copies
never queue ahead of EXPs on the ACT engine), with leftovers draining after
the pair; both score pairs of a key tile are emitted before PV(kt-1) to
double the EXP->PV slack; all weight loads are split so each chain's slices
land progressively.
"""

from contextlib import ExitStack

import numpy as np
import ml_dtypes

import concourse.bass as bass
from concourse import bacc
import concourse.tile as tile
import concourse.mybir as mybir
from concourse.bass_utils import run_bass_kernel_spmd

F32 = mybir.dt.float32
F32R = mybir.dt.float32r
BF16 = mybir.dt.bfloat16
EXP = mybir.ActivationFunctionType.Exp

N_CORES = 8
N_HEADS = 32
N_KV_HEADS = 8
D = 128          # head dim
HID = 4096
B = 2
S = 2048
T = B * S        # 4096 tokens
ROPE_BASE = 10000.0

HL = N_HEADS // N_CORES          # 4 local Q heads per core

TC = 512                         # token chunk for the QKV projection phase
QC = 256                         # query chunk in attention
N_HT = HID // 128                # 32 hidden tiles
N_QC = S // QC                   # 8 query chunks per batch
N_JT = HID // 128                # 32 output-column tiles
N_CK = T // QC                   # 16 query chunks overall


def _emit(tc_ctx, xt, wqkvt, wot, ropes, out_t):
    nc = tc_ctx.nc
    n_ch = T // TC               # 8 qkv chunks
    n_kt = S // 128              # 16 k-tiles per batch

    with ExitStack() as es:
        const_pool = es.enter_context(tc_ctx.tile_pool(name="const", bufs=1))
        # All-ones stationary: one matmul both sums over the key partition
        # axis and broadcasts the sums across all 128 partitions.
        ones_mat = const_pool.tile([128, 128], F32R)
        # Diagonal causal masks for a packed 4-slot pt tile:
        # mask4[d_off][k, slot, q] = 1.0 iff q - k - 128*d_off >= 0.
        mask4 = [const_pool.tile([128, 4, QC], BF16, name=f"mask4_{d_off}")
                 for d_off in range(2)]
        for d_off in range(2):
            nc.vector.memset(mask4[d_off], 1.0)
        # memset on a float32r tile fails the ISA check; copy from the
        # all-ones bf16 tile instead.
        nc.vector.tensor_copy(ones_mat, mask4[0][:, 0, 0:128])
        for d_off in range(2):
            for slot in range(4):
                nc.gpsimd.affine_select(
                    out=mask4[d_off][:, slot, :],
                    in_=mask4[d_off][:, slot, :],
                    compare_op=mybir.AluOpType.is_ge,
                    fill=0.0,
                    base=-128 * d_off,
                    pattern=[[1, QC]],
                    channel_multiplier=-1,
                )
        # Warm the ACT exp table before attention needs it.
        act_warm = const_pool.tile([128, 1], F32)
        nc.scalar.activation(act_warm, ones_mat[:, 0:1], EXP)

        # Q/K/V live entirely in SBUF (written by phase A, read by attention).
        qpool = es.enter_context(tc_ctx.tile_pool(name="p2_q", bufs=4))
        kvpool = es.enter_context(tc_ctx.tile_pool(name="p2_kv", bufs=2))
        ps_s = es.enter_context(
            tc_ctx.tile_pool(name="p2_ps_s", bufs=2, space="PSUM"))
        ps_o = es.enter_context(
            tc_ctx.tile_pool(name="p2_ps_o", bufs=1, space="PSUM"))
        ps_op = es.enter_context(
            tc_ctx.tile_pool(name="p3_ps", bufs=2, space="PSUM"))
        ptpool = es.enter_context(tc_ctx.tile_pool(name="p2_pt", bufs=6))
        cspool = es.enter_context(tc_ctx.tile_pool(name="p2_cs", bufs=2))
        mpool = es.enter_context(tc_ctx.tile_pool(name="p2_misc", bufs=1))
        atpool = es.enter_context(tc_ctx.tile_pool(name="p2_attn", bufs=2))
        respool = es.enter_context(tc_ctx.tile_pool(name="p3_res", bufs=6))
        wopool = es.enter_context(tc_ctx.tile_pool(name="p3_wo", bufs=1))

        # o-proj weights for this core's 512 attention rows, all 4096 cols.
        # (Loaded inside phase A, AFTER the qkv weights: this 4MB transfer
        # ahead of them was delaying the very first matmul by ~10us.)
        wo_sb = wopool.tile([128, HL, N_JT, 128], BF16)

        kvq = {}
        qtiles = {}
        attn_tiles = {}

        def alloc_q(b, qc):
            q_t = qpool.tile([128, HL, QC], BF16, tag="q",
                             name=f"q_t{b}_{qc}")
            qtiles[(b, qc)] = q_t
            return q_t

        def alloc_kv(b):
            k_sb = kvpool.tile([128, S], BF16, tag="k", name=f"k_sb{b}")
            v_sb = kvpool.tile([128, n_kt, 128], BF16, tag="v",
                               name=f"v_sb{b}")
            kvq[b] = (k_sb, v_sb)

        def emit_attn(b, qc, at_pair, sub, filler=None, prologue=None):
            k_sb, v_sb = kvq[b]
            q_sb = qtiles.pop((b, qc))
            kt_max = 2 * qc + 2
            pso = [ps_o.tile([128, 2, QC], F32, tag=f"pso{hp}",
                             name=f"pso{hp}_{b}_{qc}")
                   for hp in range(2)]
            colsum = cspool.tile([128, 4, QC], F32R)
            pts = {}

            def emit_scores(kt, hp):
                d_off = kt - 2 * qc
                if d_off == 1:
                    # Last key tile: only the upper half of the query chunk
                    # is causally visible -- compute at half width.
                    ps = ps_s.tile([128, 2, 128], F32, tag="ps",
                                   name=f"psh_{qc}_{kt}_{hp}")
                    nc.tensor.matmul(
                        ps,
                        lhsT=k_sb[:, kt * 128:(kt + 1) * 128],
                        rhs=q_sb[:, hp * 2:hp * 2 + 2, 128:QC],
                        start=True,
                        stop=True,
                    )
                    pt = ptpool.tile([128, 2, 128], BF16, tag="pt",
                                     name=f"pth_{qc}_{kt}_{hp}")
                    nc.scalar.activation(pt, ps, EXP)
                    # mask: q' >= k, i.e. the d_off=0 mask's first 128 cols.
                    nc.vector.tensor_mul(
                        pt, pt, mask4[0][:, 2 * hp:2 * hp + 2, 0:128]
                    )
                    return pt
                # One N=512 matmul covers both heads of the pair.
                ps = ps_s.tile([128, 2, QC], F32, tag="ps")
                nc.tensor.matmul(
                    ps,
                    lhsT=k_sb[:, kt * 128:(kt + 1) * 128],
                    rhs=q_sb[:, hp * 2:hp * 2 + 2, :],
                    start=True,
                    stop=True,
                )
                pt = ptpool.tile([128, 2, QC], BF16, tag="pt")
                nc.scalar.activation(pt, ps, EXP)
                if d_off == 0:
                    nc.vector.tensor_mul(
                        pt, pt, mask4[0][:, 2 * hp:2 * hp + 2, :]
                    )
                return pt

            def emit_pv(kt):
                half = (kt == kt_max - 1 and kt_max >= 2)
                for hp in range(2):
                    pt = pts.pop(kt)[hp] if hp == 1 else pts[kt][hp]
                    if half:
                        # Half-width PV into the upper query columns; the
                        # full-width accumulation group was closed at
                        # kt_max-2, so skip the group check.
                        nc.tensor.matmul(
                            pso[hp][:, :, 128:QC],
                            lhsT=v_sb[:, kt, :],
                            rhs=pt,
                            start=False,
                            stop=True,
                            skip_group_check=True,
                        )
                        nc.vector.tensor_add(
                            colsum[:, hp * 2:hp * 2 + 2, 128:QC],
                            colsum[:, hp * 2:hp * 2 + 2, 128:QC],
                            pt,
                        )
                        continue
                    nc.tensor.matmul(
                        pso[hp],
                        lhsT=v_sb[:, kt, :],
                        rhs=pt,
                        start=(kt == 0),
                        stop=(kt == max(kt_max - 2, 0)),
                    )
                    if kt == 0:
                        nc.vector.tensor_copy(
                            colsum[:, hp * 2:hp * 2 + 2, :], pt
                        )
                    else:
                        nc.vector.tensor_add(
                            colsum[:, hp * 2:hp * 2 + 2, :],
                            colsum[:, hp * 2:hp * 2 + 2, :],
                            pt,
                        )

            for kt in range(kt_max):
                # Emit both score pairs of kt before PV(kt-1) so the
                # in-order PE stream gives every EXP two matmuls of slack.
                cur = [emit_scores(kt, 0), emit_scores(kt, 1)]
                if kt >= 1:
                    emit_pv(kt - 1)
                pts[kt] = cur
                if prologue is not None and kt == 1:
                    # The previous query chunk's finalize (ones-matmul /
                    # reciprocal / normalize) runs here, hidden behind this
                    # chunk's independent score stream.
                    prologue()
                    prologue = None
                if filler is not None and kt % 2 == 1:
                    # PE filler (o-proj jt blocks of the previous pair)
                    # while the serial EXP chain catches up.
                    filler()
            emit_pv(kt_max - 1)

            # Partition-reduce + broadcast the denominators (into the score
            # PSUM pool -- scores are drained by now), then normalize.
            def finalize():
                sums_bc = [ps_s.tile([128, 2, QC], F32, tag="ps",
                                     name=f"sums{hp}_{b}_{qc}")
                           for hp in range(2)]
                for hp in range(2):
                    nc.tensor.matmul(
                        sums_bc[hp],
                        lhsT=ones_mat,
                        rhs=colsum[:, hp * 2:hp * 2 + 2, :],
                        start=True,
                        stop=True,
                    )
                recip = mpool.tile([128, 4, QC], F32, tag="recip")
                for hp in range(2):
                    # Fast single-instruction reciprocal (~12-bit): plenty
                    # for a softmax denominator, halves the DVE latency.
                    nc.vector.reciprocal_approx_fast(
                        recip[:, hp * 2:hp * 2 + 2, :], sums_bc[hp])
                for hp in range(2):
                    nc.vector.tensor_mul(
                        at_pair[:, hp * 2:hp * 2 + 2, sub, :],
                        pso[hp], recip[:, hp * 2:hp * 2 + 2, :]
                    )

            return finalize

        def emit_oproj_jt(pair, at_pair, jt, act_ok=True):
            if True:
                # Alternate PSUM pools: a 4-deep rotation so the matmul
                # chain never WAR-waits on the evacuation copy two jts back.
                # The first four jts stay on ps_op: the ps_s rotation still
                # holds sums_bc, whose reciprocal read is slow.
                if jt % 2 == 0 or jt < 4:
                    po = ps_op.tile([128, 2, QC], F32, tag="po",
                                    name=f"po_{pair}_{jt}")
                else:
                    po = ps_s.tile([128, 2, QC], F32, tag="ps",
                                   name=f"po_{pair}_{jt}")
                for hh in range(HL):
                    nc.tensor.matmul(
                        po,
                        lhsT=wo_sb[:, hh, jt, :],
                        rhs=at_pair[:, hh],
                        start=(hh == 0),
                        stop=(hh == HL - 1),
                    )
                res = respool.tile([128, 2, QC], F32, tag="res",
                                   name=f"res_{pair}_{jt}")
                # Alternate the PSUM evacuation between DVE and ACT (the ACT
                # engine is idle during o-proj segments -- but when this jt is
                # filler inside an attention pair, ACT is pacing the EXPs and
                # must not be given copies).
                if jt % 2 == 0 or not act_ok:
                    nc.vector.tensor_copy(res, po)
                else:
                    nc.scalar.activation(
                        res, po, mybir.ActivationFunctionType.Copy)
                # 2KB lines: [128 j, 2 chunks, 256 t] fp32 per write,
                # alternating queues so the 32-write burst doesn't delay the
                # next chunk's x/rope loads behind it on the sync ring.
                eng = nc.sync if jt % 2 == 0 else nc.scalar
                eng.dma_start(
                    out=out_t.ap()[jt, :, 2 * pair:2 * pair + 2, :],
                    in_=res,
                )

        def emit_oproj_pair(pair):
            # Partial o-proj for this pair's 512 tokens, contracted over the
            # core's 512 attention rows (4 head-tiles x 2 query chunks).
            at_pair = attn_tiles.pop(pair)
            for jt in range(N_JT):
                emit_oproj_jt(pair, at_pair, jt)

        def emit_attn_pair(pair, filler=None):
            b, p = divmod(pair, 4)
            at_pair = atpool.tile([128, HL, 2, QC], BF16, tag="attn",
                                  name=f"attn_{pair}")
            attn_tiles[pair] = at_pair
            fin0 = emit_attn(b, 2 * p, at_pair, 0, filler)
            fin1 = emit_attn(b, 2 * p + 1, at_pair, 1, filler,
                             prologue=fin0)
            return fin1

        # ------ Phase A: QKV projection + RoPE, attention + o-proj
        # interleaved one pair behind -------
        with tc_ctx.tile_pool(name="p1_w", bufs=1) as wpool, \
             tc_ctx.tile_pool(name="p1_x", bufs=3) as xpool, \
             tc_ctx.tile_pool(name="p1_rope", bufs=2) as rpool, \
             tc_ctx.tile_pool(name="p1_ps", bufs=2, space="PSUM") as pspool, \
             tc_ctx.tile_pool(name="p1_sh", bufs=1) as shpool:
            wq_sb = wpool.tile([128, HL + 2, N_HT, 128], BF16)
            for ot in range(HL + 2):
                # Weights on the scalar-engine queue so the first X chunk
                # (sync queue) lands in parallel. Split ot=0 finely so the
                # very first matmuls start as soon as a slice arrives.
                if ot <= 1:
                    for hq in range(4):
                        nc.scalar.dma_start(
                            out=wq_sb[:, ot, hq * 8:(hq + 1) * 8],
                            in_=wqkvt.ap()[:, ot, hq * 8:(hq + 1) * 8],
                        )
                else:
                    for hq in range(2):
                        nc.scalar.dma_start(
                            out=wq_sb[:, ot, hq * 16:(hq + 1) * 16],
                            in_=wqkvt.ap()[:, ot, hq * 16:(hq + 1) * 16],
                        )
            # o-proj weights after all qkv weights; first needed ~chunk 2.
            nc.scalar.dma_start(out=wo_sb, in_=wot.ap())
            for ch in range(n_ch):
                b, p = divmod(ch, S // TC)
                if p == 0:
                    alloc_kv(b)
                q_lo = alloc_q(b, 2 * p)
                q_hi = alloc_q(b, 2 * p + 1)
                # x in two 16-ht halves (16KB/partition each, triple
                # buffered) to fit SBUF alongside the o-proj weights.
                x_half = []
                for half in range(2):
                    xh = xpool.tile([128, N_HT // 2, TC], BF16, tag="x",
                                    name=f"x_{ch}_{half}")
                    lo = half * (N_HT // 2)
                    if ch == 0 and half == 0:
                        for piece in range(2):
                            nc.sync.dma_start(
                                out=xh[:, piece * 8:(piece + 1) * 8, :],
                                in_=xt.ap()[:, ch,
                                            lo + piece * 8:lo + (piece + 1) * 8,
                                            :],
                            )
                    else:
                        nc.sync.dma_start(out=xh,
                                          in_=xt.ap()[:, ch, lo:lo + 16, :])
                    x_half.append(xh)

                def x_sl(h):
                    return x_half[h // 16][:, h % 16, :]

                rope_sb = rpool.tile([128, 4, TC], BF16)
                nc.sync.dma_start(out=rope_sb, in_=ropes.ap()[:, ch])
                for ot in range(HL + 2):
                    if ot == HL + 1:
                        # V head, computed TRANSPOSED ([token, d]) by making
                        # the x slice the stationary operand: no HBM round
                        # trip and no DMA-transpose instructions.
                        psv = pspool.tile([128, 4, 128], F32, tag="ps")
                        for sub in range(4):
                            for h in range(N_HT):
                                nc.tensor.matmul(
                                    psv[:, sub, :],
                                    lhsT=x_sl(h)[:, sub * 128:(sub + 1) * 128],
                                    rhs=wq_sb[:, ot, h, :],
                                    start=(sub == 0 and h == 0),
                                    stop=(sub == 3 and h == N_HT - 1),
                                )
                        v_sb = kvq[b][1]
                        nc.vector.tensor_copy(
                            v_sb[:, 4 * p:4 * p + 4, :], psv
                        )
                        continue
                    ps = pspool.tile([128, TC], F32, tag="ps")
                    for h in range(N_HT):
                        nc.tensor.matmul(
                            ps,
                            lhsT=wq_sb[:, ot, h, :],
                            rhs=x_sl(h),
                            start=(h == 0),
                            stop=(h == N_HT - 1),
                        )
                    # RoPE for Q (ot<HL, scaled tables) and K (ot==HL),
                    # written straight into the SBUF attention tiles.
                    ci = 0 if ot < HL else 2
                    # sh = rotate_half(ps) * sin  (sign folded into sin)
                    sh = shpool.tile([128, TC], F32, tag="sh")
                    nc.vector.tensor_mul(
                        sh[0:64, :], ps[64:128, :], rope_sb[0:64, ci + 1, :]
                    )
                    nc.vector.tensor_mul(
                        sh[64:128, :], ps[0:64, :],
                        rope_sb[64:128, ci + 1, :]
                    )
                    tmp = shpool.tile([128, TC], F32, tag="tmp")
                    nc.vector.tensor_mul(tmp, ps, rope_sb[:, ci, :])
                    if ot < HL:
                        nc.vector.tensor_add(
                            q_lo[:, ot, :], tmp[:, 0:QC], sh[:, 0:QC]
                        )
                        nc.vector.tensor_add(
                            q_hi[:, ot, :], tmp[:, QC:TC], sh[:, QC:TC]
                        )
                    else:
                        k_sb_b = kvq[b][0]
                        nc.vector.tensor_add(
                            k_sb_b[:, p * TC:(p + 1) * TC], tmp, sh
                        )
                # Chunk ch complete: run attention for the pair that
                # became ready one chunk ago, feeding the 2-back pair's
                # o-proj jt-blocks in as PE filler inside the (ACT-bound)
                # EXP stretches; leftovers drain right after.
                if ch >= 2:
                    at_fill = attn_tiles.pop(ch - 2)
                    fqs = list(range(N_JT))

                    def fill_fn(pair=ch - 2, at=at_fill, fqs=fqs):
                        if fqs:
                            emit_oproj_jt(pair, at, fqs.pop(0), act_ok=False)

                    fin = emit_attn_pair(ch - 1, filler=fill_fn)
                    for _ in range(2):
                        if fqs:
                            emit_oproj_jt(ch - 2, at_fill, fqs.pop(0))
                    fin()
                    while fqs:
                        emit_oproj_jt(ch - 2, at_fill, fqs.pop(0))
                elif ch >= 1:
                    emit_attn_pair(ch - 1)()

        # -------- Phase B: the last pair, with the pending pair's o-proj
        # jt-blocks as PE filler inside its ACT-bound attention ----------
        at_prev = attn_tiles.pop(n_ch - 2)
        fq = list(range(N_JT))

        def filler():
            if fq:
                emit_oproj_jt(n_ch - 2, at_prev, fq.pop(0), act_ok=False)

        fin_last = emit_attn_pair(n_ch - 1, filler=filler)
        for _ in range(2):
            if fq:
                emit_oproj_jt(n_ch - 2, at_prev, fq.pop(0))
        fin_last()
        while fq:
            emit_oproj_jt(n_ch - 2, at_prev, fq.pop(0))
        emit_oproj_pair(n_ch - 1)


def _build_program():
    nc = bacc.Bacc("TRN2", target_bir_lowering=False, debug=False,
                   num_devices=N_CORES)
    xt = nc.declare_dram_parameter("xt", [128, T // TC, N_HT, TC], BF16,
                                   isOutput=False)
    wqkvt = nc.declare_dram_parameter("wqkvt", [128, HL + 2, N_HT, 128], BF16,
                                      isOutput=False)
    wot = nc.declare_dram_parameter("wot", [128, HL, N_JT, 128], BF16,
                                    isOutput=False)
    ropes = nc.declare_dram_parameter("ropes", [128, T // TC, 4, TC], BF16,
                                      isOutput=False)
    # fp32 partial o-proj: [jt, j-in-tile, query-chunk, t]; host sums cores.
    out_t = nc.declare_dram_parameter("out_t", [N_JT, 128, N_CK, QC], F32,
                                      isOutput=True)

    with tile.TileContext(nc) as tc_ctx:
        _emit(tc_ctx, xt, wqkvt, wot, ropes, out_t)
    nc.finalize()
    return nc


def _host_inputs(hidden_states, w_qkv, w_o):
    """Shard + transpose inputs for the 8 cores; returns in_maps."""
    X = np.asarray(hidden_states, dtype=np.float32).reshape(T, HID)
    # [p, ch, ht, tc] tiled layout so every DMA line is contiguous.
    xt = np.ascontiguousarray(
        X.reshape(T // TC, TC, N_HT, 128).transpose(3, 0, 2, 1)
    ).astype(ml_dtypes.bfloat16)

    # RoPE tables in [d, t] layout with rotate-half sign folded into sin and
    # the attention scale folded into the Q tables.
    inv_freq = 1.0 / (ROPE_BASE ** (np.arange(0, D, 2, dtype=np.float32) / D))
    pos = np.arange(S, dtype=np.float32)
    freqs = np.outer(pos, inv_freq)                      # (S, D/2)
    emb = np.concatenate([freqs, freqs], axis=-1)        # (S, D)
    cos = np.cos(emb).T.astype(np.float32)               # (D, S)
    sin = np.sin(emb).T.astype(np.float32)
    sgn = np.concatenate([-np.ones(D // 2), np.ones(D // 2)]).astype(np.float32)
    sins = sgn[:, None] * sin
    cos_t = np.tile(cos, (1, B))                         # (D, T)
    sins_t = np.tile(sins, (1, B))
    scale = np.float32(D ** -0.5)
    ropes = np.stack([cos_t * scale, sins_t * scale, cos_t, sins_t], axis=0)
    ropes = np.ascontiguousarray(
        ropes.reshape(4, 128, T // TC, TC).transpose(1, 2, 0, 3)
    ).astype(ml_dtypes.bfloat16)

    w_qkv = np.asarray(w_qkv, dtype=np.float32)
    w_o = np.asarray(w_o, dtype=np.float32)
    q_sz = N_HEADS * D
    kv_sz = N_KV_HEADS * D
    in_maps = []
    for c in range(N_CORES):
        qr = w_qkv[c * HL * D:(c + 1) * HL * D]
        kr = w_qkv[q_sz + c * D:q_sz + (c + 1) * D]
        vr = w_qkv[q_sz + kv_sz + c * D:q_sz + kv_sz + (c + 1) * D]
        w_shard = np.concatenate([qr, kr, vr], axis=0)           # (768, HID)
        wqkvt_c = np.ascontiguousarray(
            w_shard.reshape(HL + 2, 128, N_HT, 128).transpose(3, 0, 2, 1)
        ).astype(ml_dtypes.bfloat16)
        # o-proj slice: this core's 512 attention rows, all 4096 columns,
        # laid out [d-part, hh, jt, j].
        wo_rows = w_o[:, c * HL * D:(c + 1) * HL * D]            # (4096, 512)
        wot_c = np.ascontiguousarray(
            wo_rows.T.reshape(HL, 128, N_JT, 128).transpose(1, 0, 2, 3)
        ).astype(ml_dtypes.bfloat16)
        in_maps.append({
            "xt": xt, "wqkvt": wqkvt_c, "wot": wot_c, "ropes": ropes,
        })
    return in_maps


def _run(hidden_states, w_qkv, w_o, trace=False, tmpdir=None):
    in_maps = _host_inputs(hidden_states, w_qkv, w_o)
    nc = _build_program()
    res = run_bass_kernel_spmd(nc, in_maps, list(range(N_CORES)),
                               trace=trace, tmpdir=tmpdir)
    acc = np.zeros((N_JT, 128, N_CK, QC), dtype=np.float32)
    for c in range(N_CORES):
        acc += np.asarray(res.results[c]["out_t"])
    # [jt, j, ck, t] -> [j(4096), t(4096)] -> [B, S, HID]
    out_jt = acc.reshape(N_JT * 128, N_CK * QC)
    out = np.ascontiguousarray(out_jt.T).reshape(B, S, HID).astype(np.float32)
    return out, res


def kernel(hidden_states, w_qkv, w_o):
    out, _ = _run(hidden_states, w_qkv, w_o, trace=False)
    return out
